# revision 35
# baseline (speedup 1.0000x reference)
"""CrossAttention kernel for 8 Trainium2 NeuronCores.

Problem (hardcoded): B=8, SQ=SK=1024, Q_DIM=2048, KV_DIM=1024, E_DIM=2048,
H=16 heads, HD=128.  out = softmax((X_q Wq^T + bq)(X_k Wk^T + bk)^T / sqrt(HD))
                            @ (X_v Wv^T + bv) @ Wo^T + bo

Sharding: data-parallel over batch - each of the 8 cores computes one batch
element end-to-end; no collectives.

Per-core dataflow:
  - All four projections run as fp8 DoubleRow matmuls (K=256 per
    instruction, 0.5 cyc/row) with a 3-term hi/lo split:
        x*w = x_hi*w_hi + x_lo*w_hi + x_hi*w_lo   (lo*lo dropped)
    hi terms are e4m3, lo terms e5m2.  Activations are pre-scaled by 16 and
    weights by 32 (host side) so the splits stay in fp8's normal range; the
    1/512 is folded into the PSUM drain (DVE tensor_scalar mult+bias-add).
  - Q/K projections produce qT/kT in [e_part, s] fp16; V is produced
    directly in [s_part, e] fp16 (stationary=x, moving=w) so no transpose.
    bv is folded into bo on the host (softmax weights sum to 1).
  - Per head: scoresT[sk,sq] = kT_h.T @ qT_h (fp16, f32 PSUM);
    P = exp(scoresT) on ACT -> fp16; o[sq, 129] = sum_sk P.T @ v[sk, h||1/16]
    (col 128 = rowsum/16); ao16[s,e] = o * (16/rowsum) -> fp16 (= 16*ao).
    Q-proj for head h+1 is interleaved into the head loop so exp overlaps PE.
  - ao16 is DMA-transposed per (sq-tile, 8-head half) as soon as ready, then
    split on DVE into e4m3 hi + e5m2 lo for the DoubleRow out-projection
    (Wo streamed per 512-col slice, double buffered); drain scale 1/512.
"""

import sys

sys.path.insert(0, "/opt/trn_rl_repo")

import numpy as np
import ml_dtypes

import concourse.tile as tile
from concourse import bacc
import concourse.mybir as mybir
from concourse.bass_utils import run_bass_kernel_spmd

F32 = mybir.dt.float32
F16 = mybir.dt.float16
E4 = mybir.dt.float8e4
E5 = mybir.dt.float8e5
DR = mybir.MatmulPerfMode.DoubleRow
ACT_IDENT = mybir.ActivationFunctionType.Identity
ACT_EXP = mybir.ActivationFunctionType.Exp
MULT = mybir.AluOpType.mult
ADD = mybir.AluOpType.add

ML_E4 = ml_dtypes.float8_e4m3
ML_E5 = ml_dtypes.float8_e5m2

B = 8
S = 1024          # SQ == SK
DQ = 2048         # query input dim
DKV = 1024        # key/value input dim
E = 2048          # embed dim
H = 16            # heads
HD = 128          # head dim
VROW = HD + 1     # head block in v_sb incl. the 1/16 column
NCQ = DQ // 256   # 8 chunk-pairs for Q contraction
NCK = DKV // 256  # 4 chunk-pairs for K/V contraction
NCE = E // 256    # 8 chunk-pairs for out-proj contraction
XS = 16.0         # host scale on activations
WS = 32.0         # host scale on weights
INV = 1.0 / (XS * WS)

_CACHED = {}


def _build():
    nc = bacc.Bacc("TRN2", target_bir_lowering=False, debug=False)

    xq_hi = nc.dram_tensor("xq_hi", [128, NCQ, 2, S], E4, kind="ExternalInput")
    xq_lo = nc.dram_tensor("xq_lo", [128, NCQ, 2, S], E5, kind="ExternalInput")
    xk_hi = nc.dram_tensor("xk_hi", [128, NCK, 2, S], E4, kind="ExternalInput")
    xk_lo = nc.dram_tensor("xk_lo", [128, NCK, 2, S], E5, kind="ExternalInput")
    xv_hi = nc.dram_tensor("xv_hi", [128, NCK, 2, S], E4, kind="ExternalInput")
    xv_lo = nc.dram_tensor("xv_lo", [128, NCK, 2, S], E5, kind="ExternalInput")
    # weight streams; leading stream-slice dim for contiguous per-slice DMA
    wq_hi = nc.dram_tensor("wq_hi", [128, 8, NCQ, 2, 256], E4, kind="ExternalInput")
    wq_lo = nc.dram_tensor("wq_lo", [128, 8, NCQ, 2, 256], E5, kind="ExternalInput")
    wk_hi = nc.dram_tensor("wk_hi", [128, 8, NCK, 2, 256], E4, kind="ExternalInput")
    wk_lo = nc.dram_tensor("wk_lo", [128, 8, NCK, 2, 256], E5, kind="ExternalInput")
    wv_hi = nc.dram_tensor("wv_hi", [128, 4, NCK, 2, 512], E4, kind="ExternalInput")
    wv_lo = nc.dram_tensor("wv_lo", [128, 4, NCK, 2, 512], E5, kind="ExternalInput")
    wo_hi = nc.dram_tensor("wo_hi", [128, 4, NCE, 2, 512], E4, kind="ExternalInput")
    wo_lo = nc.dram_tensor("wo_lo", [128, 4, NCE, 2, 512], E5, kind="ExternalInput")
    bqk = nc.dram_tensor("bqk", [128, 2, H], F32, kind="ExternalInput")
    out = nc.dram_tensor("out", [S, E], F32, kind="ExternalOutput")

    with tile.TileContext(nc) as tc:
        with (
            tc.tile_pool(name="arena", bufs=1) as pa,
            tc.tile_pool(name="small", bufs=4) as small,
            tc.tile_pool(name="psA", bufs=2, space="PSUM") as psA,
            tc.tile_pool(name="psB", bufs=2, space="PSUM") as psB,
        ):
            # ---------------- static SBUF tiles ----------------
            xqh_sb = pa.tile([128, NCQ, 2, S], E4, tag="xqh")
            xql_sb = pa.tile([128, NCQ, 2, S], E5, tag="xql")
            xkh_sb = pa.tile([128, NCK, 2, S], E4, tag="xkh")
            xkl_sb = pa.tile([128, NCK, 2, S], E5, tag="xkl")
            xvh_sb = pa.tile([128, NCK, 2, S], E4, tag="xvh")
            xvl_sb = pa.tile([128, NCK, 2, S], E5, tag="xvl")
            qT_sb = [pa.tile([128, 8, S], F16, tag=f"qT{j}", name=f"qT{j}")
                     for j in range(2)]
            # kT(h) and ao(h, :) share bytes: kT(h) is dead once scores(h)
            # ran, and ao[h-block] is written later the same iteration.
            # Layout [p, s-block, h, 128] keeps both the scores stationary
            # slices and the (sq, head-half) transpose inputs contiguous.
            ktao_sb = pa.tile([128, 8, H, 128], F16, tag="ktao")
            v_sb = pa.tile([128, 8, H * VROW], F16, tag="v")
            p_sb = pa.tile([128, 8, S], F16, tag="P")
            # w stream buffers (double buffered via dim 0); slot sized for Q
            wsh_sb = pa.tile([128, 2, NCQ, 2, 256], E4, tag="wsh")
            wsl_sb = pa.tile([128, 2, NCQ, 2, 256], E5, tag="wsl")
            bqk_sb = pa.tile([128, 2, H], F32, tag="bqk")
            aot_sb = pa.tile([128, 2, 8, 128], F16, tag="aot")
            outst_sb = pa.tile([128, 2, 512], F32, tag="outst")

            def qT(h):
                return qT_sb[h // 8][:, h % 8, :]

            def kT(h, sk):
                return ktao_sb[:, sk, h, :]

            def ao(h, sq):
                return ktao_sb[:, sq, h, :]

            # ---------------- input DMAs (ordered for earliest start) ------

            # v ones-columns = 1/16 (turns rowsum into rowsum/16 so the
            # final per-row reciprocal yields 16/rowsum -> ao16 = 16*ao)
            v4 = v_sb.rearrange("p t (h c) -> p t h c", c=VROW)
            nc.vector.memset(v4[:, :, :, HD:], 1.0 / XS)

            # -------- generic hi/lo DoubleRow projection matmul set --------
            def dr_terms(ps_out, w_hi_ap, w_lo_ap, x_hi_ap, x_lo_ap, ncp,
                         wslice, xslice):
                """Emit 3*ncp DoubleRow matmuls accumulating into ps_out."""
                n = 0
                nt = 3 * ncp
                for xt, wt in ((x_hi_ap, w_hi_ap), (x_lo_ap, w_hi_ap),
                               (x_hi_ap, w_lo_ap)):
                    for c in range(ncp):
                        nc.tensor.matmul(
                            ps_out, wslice(wt, c), xslice(xt, c),
                            start=(n == 0), stop=(n == nt - 1), perf_mode=DR,
                        )
                        n += 1

            # ---------------- K projection: kT[e,s] ----------------
            # stream wk per eg (2 e-tiles); psum per e-tile [128, S]
            wkh_s = pa.tile([128, 2, NCK, 2, 256], E4, tag="wsh", name="wkh_s")
            wkl_s = pa.tile([128, 2, NCK, 2, 256], E5, tag="wsl", name="wkl_s")
            wvh_s = pa.tile([128, 2, NCK, 2, 512], E4, tag="wsh", name="wvh_s")
            wvl_s = pa.tile([128, 2, NCK, 2, 512], E5, tag="wsl", name="wvl_s")
            for eg in range(8):
                db = eg % 2
                nc.sync.dma_start(out=wkh_s[:, db], in_=wk_hi.ap()[:, eg])
                if eg == 0:
                    # first matmuls need only wk0-hi + the first xk chunks
                    nc.sync.dma_start(out=xkh_sb[:, 0:2],
                                      in_=xk_hi.ap()[:, 0:2])
                nc.sync.dma_start(out=wkl_s[:, db], in_=wk_lo.ap()[:, eg])
                # interleave the big x transfers in small chunks so they
                # don't wedge ahead of the streamed weight slices
                if eg == 0:
                    nc.sync.dma_start(out=xkh_sb[:, 2:4],
                                      in_=xk_hi.ap()[:, 2:4])
                    nc.sync.dma_start(out=xkl_sb[:, 0:2],
                                      in_=xk_lo.ap()[:, 0:2])
                    nc.sync.dma_start(out=xkl_sb[:, 2:4],
                                      in_=xk_lo.ap()[:, 2:4])
                    nc.sync.dma_start(out=bqk_sb, in_=bqk.ap())
                elif eg <= 2:
                    a = 2 * (eg - 1)
                    nc.sync.dma_start(out=xvh_sb[:, a:a + 2],
                                      in_=xv_hi.ap()[:, a:a + 2])
                    nc.sync.dma_start(out=xvl_sb[:, a:a + 2],
                                      in_=xv_lo.ap()[:, a:a + 2])
                elif eg <= 6:
                    src = (xq_hi, xq_lo)[(eg - 3) % 2]
                    dst = (xqh_sb, xql_sb)[(eg - 3) % 2]
                    a = 4 * ((eg - 3) // 2)
                    nc.sync.dma_start(out=dst[:, a:a + 4],
                                      in_=src.ap()[:, a:a + 4])
                elif eg == 7:
                    # prefetch the first V weight slice (slot bytes disjoint
                    # from the wk slots, so this does not wait on K)
                    nc.sync.dma_start(out=wvh_s[:, 1], in_=wv_hi.ap()[:, 0])
                    nc.sync.dma_start(out=wvl_s[:, 1], in_=wv_lo.ap()[:, 0])
                for t in range(2):
                    et = eg * 2 + t
                    ps = psA.tile([128, S], F32, tag="big", name=f"psk{et}")
                    for sh in range(2):
                        dr_terms(
                            ps[:, sh * 512:(sh + 1) * 512],
                            wkh_s, wkl_s, xkh_sb, xkl_sb, NCK,
                            lambda w, c, db=db, t=t: w[:, db, c, :,
                                                      t * 128:(t + 1) * 128],
                            lambda x, c, sh=sh: x[:, c, :,
                                                  sh * 512:(sh + 1) * 512],
                        )
                    nc.vector.tensor_scalar(
                        out=ktao_sb[:, :, et, :], in0=ps, scalar1=INV,
                        scalar2=bqk_sb[:, 1, et:et + 1], op0=MULT, op1=ADD,
                    )

            # ---------------- V projection: v[s, e] (no bias) ----------------
            # first Q weight slice staged in the (still idle) outst/aot slots
            wq0h = pa.tile([128, NCQ, 2, 256], E4, tag="outst", name="wq0h")
            wq0l = pa.tile([128, NCQ, 2, 256], E5, tag="aot", name="wq0l")
            for eq in range(4):
                db = (eq + 1) % 2
                if eq > 0:
                    nc.sync.dma_start(out=wvh_s[:, db], in_=wv_hi.ap()[:, eq])
                    nc.sync.dma_start(out=wvl_s[:, db], in_=wv_lo.ap()[:, eq])
                if eq == 1:
                    nc.sync.dma_start(out=wq0h, in_=wq_hi.ap()[:, 0])
                    nc.sync.dma_start(out=wq0l, in_=wq_lo.ap()[:, 0])
                for st in range(8):
                    # psB so V's pipeline does not wait on K's kT drains
                    ps = psB.tile([128, 512], F32, tag="sc", name=f"psv{eq}_{st}")
                    # roles swapped vs K/Q: stationary = x chunk, moving = w
                    dr_terms(
                        ps, xvh_sb, xvl_sb, wvh_s, wvl_s, NCK,
                        lambda x, c, st=st: x[:, c, :,
                                             st * 128:(st + 1) * 128],
                        lambda w, c, db=db: w[:, db, c],
                    )
                    nc.vector.tensor_scalar_mul(
                        v4[:, st, 4 * eq:4 * eq + 4, 0:HD], ps, INV)

            # ---------------- merged Q projection + attention ----------------
            aoThi = [pa.tile([128, 4, H, 128], E4, tag=t, name=f"aoThi{j}")
                     for j, t in ((0, "xkh"), (1, "xkl"))]
            aoTlo = [pa.tile([128, 4, H, 128], E5, tag=t, name=f"aoTlo{j}")
                     for j, t in ((0, "xvh"), (1, "xvl"))]

            def proj_q(h):
                # slice s>=1 lives in slot (s+1)%2; prefetch the next slice a
                # full head-pair (~24us) before its first use.  Slice 0 was
                # staged in wq0h/wq0l during the V projection.
                s = h // 2
                if h % 2 == 0 and s + 1 < 8:
                    nc.sync.dma_start(out=wsh_sb[:, s % 2],
                                      in_=wq_hi.ap()[:, s + 1])
                    nc.sync.dma_start(out=wsl_sb[:, s % 2],
                                      in_=wq_lo.ap()[:, s + 1])
                t = h % 2
                if s == 0:
                    whi, wlo = wq0h, wq0l
                    wsl_fn = (lambda w, c, t=t:
                              w[:, c, :, t * 128:(t + 1) * 128])
                else:
                    whi, wlo = wsh_sb, wsl_sb
                    wsl_fn = (lambda w, c, db=(s + 1) % 2, t=t:
                              w[:, db, c, :, t * 128:(t + 1) * 128])
                ps = psA.tile([128, S], F32, tag="big", name=f"psq{h}")
                for sh in range(2):
                    dr_terms(
                        ps[:, sh * 512:(sh + 1) * 512],
                        whi, wlo, xqh_sb, xql_sb, NCQ,
                        wsl_fn,
                        lambda x, c, sh=sh: x[:, c, :, sh * 512:(sh + 1) * 512],
                    )
                nc.vector.tensor_scalar(
                    out=qT(h), in0=ps, scalar1=INV,
                    scalar2=bqk_sb[:, 0, h:h + 1], op0=MULT, op1=ADD,
                )

            def transpose_split(sq, hf):
                db = (sq + hf) % 2
                at = aot_sb[:, db]
                nc.sync.dma_start_transpose(
                    out=at,
                    in_=ktao_sb[:, sq, hf * 8:(hf + 1) * 8, :])
                nc.vector.tensor_copy(
                    out=aoThi[sq // 4][:, sq % 4, hf * 8:(hf + 1) * 8, :],
                    in_=at)
                nc.vector.tensor_sub(
                    aoTlo[sq // 4][:, sq % 4, hf * 8:(hf + 1) * 8, :],
                    at,
                    aoThi[sq // 4][:, sq % 4, hf * 8:(hf + 1) * 8, :])

            # out-projection weight stream tiles (alias dead qT after h15)
            woh_sb = pa.tile([128, 2, NCE, 2, 512], E4, tag="qT0", name="woh")
            wol_sb = pa.tile([128, 2, NCE, 2, 512], E5, tag="qT1", name="wol")

            def wo_fetch(e2c):
                db = e2c % 2
                nc.sync.dma_start(out=woh_sb[:, db], in_=wo_hi.ap()[:, e2c])
                nc.sync.dma_start(out=wol_sb[:, db], in_=wo_lo.ap()[:, e2c])

            proj_q(0)
            for h in range(H):
                # scoresT[sk, sq] then P = exp on ACT
                for sk in range(8):
                    ssp = psB.tile([128, S], F32, tag="sc", name=f"ss{h}_{sk}")
                    for sh in range(2):
                        nc.tensor.matmul(
                            ssp[:, sh * 512:(sh + 1) * 512],
                            kT(h, sk),
                            qT(h)[:, sh * 512:(sh + 1) * 512],
                            start=True, stop=True,
                        )
                    nc.scalar.activation(
                        out=p_sb[:, sk, :], in_=ssp, func=ACT_EXP,
                        bias=0.0, scale=1.0,
                    )
                # next head's Q projection overlaps the exp
                if h + 1 < H:
                    proj_q(h + 1)
                if h == 15:
                    wo_fetch(0)
                # attention-weighted V (+ rowsum/16 via the 1/16 column)
                for sq in range(8):
                    op = psB.tile([128, 512], F32, tag="sc", name=f"op{h}_{sq}")
                    for sk in range(8):
                        nc.tensor.matmul(
                            op[:, 0:VROW],
                            p_sb[:, sk, sq * 128:(sq + 1) * 128],
                            v_sb[:, sk, h * VROW:(h + 1) * VROW],
                            start=(sk == 0), stop=(sk == 7),
                        )
                    recip = small.tile([128, 1], F32, tag="recip")
                    nc.vector.reciprocal(out=recip, in_=op[:, HD:VROW])
                    nc.vector.tensor_scalar_mul(
                        ao(h, sq), op[:, 0:HD], recip)
                    # heads 0-7 half: one transpose+split per head (spreads
                    # the DVE load); heads 8-15 half: all at the last head
                    if h >= 7 and sq == h - 7:
                        transpose_split(sq, 0)
                    if h == 15:
                        transpose_split(sq, 1)

            # ---------------- out projection ----------------
            for e2c in range(4):
                db = e2c % 2
                if e2c + 1 < 4:
                    wo_fetch(e2c + 1)
                for sq in range(8):
                    ps = psA.tile([128, 512], F32, tag="big",
                                  name=f"pso{e2c}_{sq}")
                    n = 0
                    for at, wt in ((aoThi, woh_sb), (aoTlo, woh_sb),
                                   (aoThi, wol_sb)):
                        for c in range(NCE):
                            nc.tensor.matmul(
                                ps,
                                at[sq // 4][:, sq % 4, 2 * c:2 * c + 2, :],
                                wt[:, db, c],
                                start=(n == 0), stop=(n == 23),
                                perf_mode=DR,
                            )
                            n += 1
                    ob = outst_sb[:, sq % 2]
                    nc.scalar.activation(
                        out=ob, in_=ps, func=ACT_IDENT, bias=0.0,
                        scale=INV,
                    )
                    nc.scalar.dma_start(
                        out=out.ap()[sq * 128:(sq + 1) * 128,
                                     e2c * 512:(e2c + 1) * 512],
                        in_=ob,
                    )

    nc.compile()
    return nc


def _get_nc():
    if "nc" not in _CACHED:
        _CACHED["nc"] = _build()
    return _CACHED["nc"]


def _split8(a, scale):
    """a (f32) -> (hi e4m3, lo e5m2) of a*scale."""
    s = (a * np.float32(scale)).astype(np.float32)
    hi = s.astype(ML_E4)
    lo = (s - hi.astype(np.float32)).astype(ML_E5)
    return hi, lo


def _wlayout(wT, npair, nslice, width):
    """[K, E] -> [128, nslice, npair, 2, width] stream layout."""
    K, Eo = wT.shape
    w = wT.reshape(npair, 2, 128, nslice, width)
    return np.ascontiguousarray(w.transpose(2, 3, 0, 1, 4))


def _xlayout(xT, npair):
    """[K, S] -> [128, npair, 2, S]."""
    K, Sx = xT.shape
    x = xT.reshape(npair, 2, 128, Sx)
    return np.ascontiguousarray(x.transpose(2, 0, 1, 3))


def _prepare_in_maps(inputs):
    query = np.asarray(inputs["query"], dtype=np.float32)
    key = np.asarray(inputs["key"], dtype=np.float32)
    value = np.asarray(inputs["value"], dtype=np.float32)
    Wq = np.asarray(inputs["Wq"], dtype=np.float32)
    bq = np.asarray(inputs["bq"], dtype=np.float32)
    Wk = np.asarray(inputs["Wk"], dtype=np.float32)
    bk = np.asarray(inputs["bk"], dtype=np.float32)
    Wv = np.asarray(inputs["Wv"], dtype=np.float32)
    Wo = np.asarray(inputs["Wo"], dtype=np.float32)

    scale_hd = np.float32(HD ** -0.5)
    wq_hi, wq_lo = _split8(Wq.T * scale_hd, WS)
    wk_hi, wk_lo = _split8(Wk.T, WS)
    wv_hi, wv_lo = _split8(Wv.T, WS)
    wo_hi, wo_lo = _split8(Wo.T, WS)
    wmaps = {
        "wq_hi": _wlayout(wq_hi, NCQ, 8, 256),
        "wq_lo": _wlayout(wq_lo, NCQ, 8, 256),
        "wk_hi": _wlayout(wk_hi, NCK, 8, 256),
        "wk_lo": _wlayout(wk_lo, NCK, 8, 256),
        "wv_hi": _wlayout(wv_hi, NCK, 4, 512),
        "wv_lo": _wlayout(wv_lo, NCK, 4, 512),
        "wo_hi": _wlayout(wo_hi, NCE, 4, 512),
        "wo_lo": _wlayout(wo_lo, NCE, 4, 512),
    }
    bqk = np.ascontiguousarray(
        np.stack([(bq * scale_hd), bk]).reshape(2, H, 128).transpose(2, 0, 1)
    ).astype(np.float32)

    in_maps = []
    for b in range(B):
        xqh, xql = _split8(query[b].T, XS)
        xkh, xkl = _split8(key[b].T, XS)
        xvh, xvl = _split8(value[b].T, XS)
        m = {
            "xq_hi": _xlayout(xqh, NCQ), "xq_lo": _xlayout(xql, NCQ),
            "xk_hi": _xlayout(xkh, NCK), "xk_lo": _xlayout(xkl, NCK),
            "xv_hi": _xlayout(xvh, NCK), "xv_lo": _xlayout(xvl, NCK),
            "bqk": bqk,
        }
        m.update(wmaps)
        in_maps.append(m)
    return in_maps


def run_on_device(inputs, **spmd_kwargs):
    """Run the bass kernel; returns (out [B,S,E] f32, BassKernelResults)."""
    in_maps = _prepare_in_maps(inputs)
    bo = np.asarray(inputs["bo"], dtype=np.float32)
    bv = np.asarray(inputs["bv"], dtype=np.float32)
    Wo = np.asarray(inputs["Wo"], dtype=np.float32)
    # softmax weights sum to 1, so the v bias shifts ao directly:
    # out = (ao + bv) @ Wo.T + bo
    bo_eff = bo + bv @ Wo.T
    res = run_bass_kernel_spmd(_get_nc(), in_maps,
                               core_ids=list(range(B)), **spmd_kwargs)
    out = np.stack([res.results[b]["out"] for b in range(B)], axis=0)
    return (out + bo_eff).astype(np.float32), res


def _numpy_reference(query, key, value, attention_mask,
                     Wq, bq, Wk, bk, Wv, bv, Wo, bo):
    # general fallback (only used when attention_mask isn't all ones)
    Bb, SQ, _ = query.shape
    SK = key.shape[1]
    q = query @ Wq.T + bq
    k = key @ Wk.T + bk
    v = value @ Wv.T + bv
    q = q.reshape(Bb, SQ, H, HD).transpose(0, 2, 1, 3)
    k = k.reshape(Bb, SK, H, HD).transpose(0, 2, 1, 3)
    v = v.reshape(Bb, SK, H, HD).transpose(0, 2, 1, 3)
    scores = np.einsum("bhqd,bhkd->bhqk", q, k) * (HD ** -0.5)
    scores = np.where(attention_mask[:, None, :, :] == 0,
                      np.float32(-1e10), scores)
    scores -= scores.max(-1, keepdims=True)
    p = np.exp(scores)
    p /= p.sum(-1, keepdims=True)
    o = np.einsum("bhqk,bhkd->bhqd", p, v)
    o = o.transpose(0, 2, 1, 3).reshape(Bb, SQ, E)
    return (o @ Wo.T + bo).astype(np.float32)


def kernel(**inputs):
    mask = np.asarray(inputs["attention_mask"])
    if not mask.all():
        return _numpy_reference(
            np.asarray(inputs["query"], dtype=np.float32),
            np.asarray(inputs["key"], dtype=np.float32),
            np.asarray(inputs["value"], dtype=np.float32), mask,
            np.asarray(inputs["Wq"], dtype=np.float32),
            np.asarray(inputs["bq"], dtype=np.float32),
            np.asarray(inputs["Wk"], dtype=np.float32),
            np.asarray(inputs["bk"], dtype=np.float32),
            np.asarray(inputs["Wv"], dtype=np.float32),
            np.asarray(inputs["bv"], dtype=np.float32),
            np.asarray(inputs["Wo"], dtype=np.float32),
            np.asarray(inputs["bo"], dtype=np.float32))
    out, _ = run_on_device(inputs)
    return out


# revision 41
# speedup vs baseline: 1.0295x; 1.0295x over previous
"""CrossAttention kernel for 8 Trainium2 NeuronCores.

Problem (hardcoded): B=8, SQ=SK=1024, Q_DIM=2048, KV_DIM=1024, E_DIM=2048,
H=16 heads, HD=128.  out = softmax((X_q Wq^T + bq)(X_k Wk^T + bk)^T / sqrt(HD))
                            @ (X_v Wv^T + bv) @ Wo^T + bo

Sharding: data-parallel over batch - each of the 8 cores computes one batch
element end-to-end; no collectives.

Per-core dataflow:
  - All four projections run as fp8 DoubleRow matmuls (K=256 per
    instruction, 0.5 cyc/row) with a 3-term hi/lo split:
        x*w = x_hi*w_hi + x_lo*w_hi + x_hi*w_lo   (lo*lo dropped)
    hi terms are e4m3, lo terms e5m2.  Activations are pre-scaled by 16 and
    weights by 32 (host side) so the splits stay in fp8's normal range; the
    1/512 is folded into the PSUM drain (DVE tensor_scalar mult+bias-add).
  - Q/K projections produce qT/kT in [e_part, s] fp16; V is produced
    directly in [s_part, e] fp16 (stationary=x, moving=w) so no transpose.
    bv is folded into bo on the host (softmax weights sum to 1).
  - Per head: scoresT[sk,sq] = kT_h.T @ qT_h (fp16, f32 PSUM);
    P = exp(scoresT) on ACT -> fp16; o[sq, 129] = sum_sk P.T @ v[sk, h||1/16]
    (col 128 = rowsum/16); ao16[s,e] = o * (16/rowsum) -> fp16 (= 16*ao).
    Q-proj for head h+1 is interleaved into the head loop so exp overlaps PE.
  - ao16 is DMA-transposed per (sq-tile, 8-head half) as soon as ready, then
    split on DVE into e4m3 hi + e5m2 lo for the DoubleRow out-projection
    (Wo streamed per 512-col slice, double buffered); drain scale 1/512.
"""

import sys

sys.path.insert(0, "/opt/trn_rl_repo")

import numpy as np
import ml_dtypes

import concourse.tile as tile
from concourse import bacc
import concourse.mybir as mybir
from concourse.bass_utils import run_bass_kernel_spmd

F32 = mybir.dt.float32
F16 = mybir.dt.float16
E4 = mybir.dt.float8e4
E5 = mybir.dt.float8e5
DR = mybir.MatmulPerfMode.DoubleRow
ACT_IDENT = mybir.ActivationFunctionType.Identity
ACT_EXP = mybir.ActivationFunctionType.Exp
MULT = mybir.AluOpType.mult
ADD = mybir.AluOpType.add

ML_E4 = ml_dtypes.float8_e4m3
ML_E5 = ml_dtypes.float8_e5m2

B = 8
S = 1024          # SQ == SK
DQ = 2048         # query input dim
DKV = 1024        # key/value input dim
E = 2048          # embed dim
H = 16            # heads
HD = 128          # head dim
VROW = HD + 1     # head block in v_sb incl. the 1/16 column
NCQ = DQ // 256   # 8 chunk-pairs for Q contraction
NCK = DKV // 256  # 4 chunk-pairs for K/V contraction
NCE = E // 256    # 8 chunk-pairs for out-proj contraction
XS = 16.0         # host scale on activations
WS = 32.0         # host scale on weights
INV = 1.0 / (XS * WS)

_CACHED = {}


def _build():
    nc = bacc.Bacc("TRN2", target_bir_lowering=False, debug=False)

    xq_hi = nc.dram_tensor("xq_hi", [128, NCQ, 2, S], E4, kind="ExternalInput")
    xq_lo = nc.dram_tensor("xq_lo", [128, NCQ, 2, S], E5, kind="ExternalInput")
    xk_hi = nc.dram_tensor("xk_hi", [128, NCK, 2, S], E4, kind="ExternalInput")
    xk_lo = nc.dram_tensor("xk_lo", [128, NCK, 2, S], E5, kind="ExternalInput")
    xv_hi = nc.dram_tensor("xv_hi", [128, NCK, 2, S], E4, kind="ExternalInput")
    xv_lo = nc.dram_tensor("xv_lo", [128, NCK, 2, S], E5, kind="ExternalInput")
    # weight streams; leading stream-slice dim for contiguous per-slice DMA
    wq_hi = nc.dram_tensor("wq_hi", [128, 8, NCQ, 2, 256], E4, kind="ExternalInput")
    wq_lo = nc.dram_tensor("wq_lo", [128, 8, NCQ, 2, 256], E5, kind="ExternalInput")
    wk_hi = nc.dram_tensor("wk_hi", [128, 8, NCK, 2, 256], E4, kind="ExternalInput")
    wk_lo = nc.dram_tensor("wk_lo", [128, 8, NCK, 2, 256], E5, kind="ExternalInput")
    wv_hi = nc.dram_tensor("wv_hi", [128, 4, NCK, 2, 512], E4, kind="ExternalInput")
    wv_lo = nc.dram_tensor("wv_lo", [128, 4, NCK, 2, 512], E5, kind="ExternalInput")
    wo_hi = nc.dram_tensor("wo_hi", [128, 4, NCE, 2, 512], E4, kind="ExternalInput")
    wo_lo = nc.dram_tensor("wo_lo", [128, 4, NCE, 2, 512], E5, kind="ExternalInput")
    bqk = nc.dram_tensor("bqk", [128, 2, H], F32, kind="ExternalInput")
    out = nc.dram_tensor("out", [S, E], F32, kind="ExternalOutput")

    with tile.TileContext(nc) as tc:
        with (
            tc.tile_pool(name="arena", bufs=1) as pa,
            tc.tile_pool(name="small", bufs=4) as small,
            tc.tile_pool(name="psA", bufs=2, space="PSUM") as psA,
            tc.tile_pool(name="psB", bufs=2, space="PSUM") as psB,
        ):
            # ---------------- static SBUF tiles ----------------
            xqh_sb = pa.tile([128, NCQ, 2, S], E4, tag="xqh")
            xql_sb = pa.tile([128, NCQ, 2, S], E5, tag="xql")
            xkh_sb = pa.tile([128, NCK, 2, S], E4, tag="xkh")
            xkl_sb = pa.tile([128, NCK, 2, S], E5, tag="xkl")
            xvh_sb = pa.tile([128, NCK, 2, S], E4, tag="xvh")
            xvl_sb = pa.tile([128, NCK, 2, S], E5, tag="xvl")
            qT_sb = [pa.tile([128, 8, S], F16, tag=f"qT{j}", name=f"qT{j}")
                     for j in range(2)]
            # kT(h) and ao(h, :) share bytes: kT(h) is dead once scores(h)
            # ran, and ao[h-block] is written later the same iteration.
            # Layout [p, s-block, h, 128] keeps both the scores stationary
            # slices and the (sq, head-half) transpose inputs contiguous.
            ktao_sb = pa.tile([128, 8, H, 128], F16, tag="ktao")
            v_sb = pa.tile([128, 8, H * VROW], F16, tag="v")
            p_sb = pa.tile([128, 8, S], F16, tag="P")
            # w stream buffers (double buffered via dim 0); slot sized for Q
            wsh_sb = pa.tile([128, 2, NCQ, 2, 256], E4, tag="wsh")
            wsl_sb = pa.tile([128, 2, NCQ, 2, 256], E5, tag="wsl")
            bqk_sb = pa.tile([128, 2, H], F32, tag="bqk")
            aot_sb = pa.tile([128, 2, 8, 128], F16, tag="aot")
            outst_sb = pa.tile([128, 2, 512], F32, tag="outst")

            def qT(h):
                return qT_sb[h // 8][:, h % 8, :]

            def kT(h, sk):
                return ktao_sb[:, sk, h, :]

            def ao(h, sq):
                return ktao_sb[:, sq, h, :]

            # ---------------- input DMAs (ordered for earliest start) ------

            # v ones-columns = 1/16 (turns rowsum into rowsum/16 so the
            # final per-row reciprocal yields 16/rowsum -> ao16 = 16*ao)
            v4 = v_sb.rearrange("p t (h c) -> p t h c", c=VROW)
            nc.vector.memset(v4[:, :, :, HD:], 1.0 / XS)

            # -------- generic hi/lo DoubleRow projection matmul set --------
            def dr_terms(ps_out, w_hi_ap, w_lo_ap, x_hi_ap, x_lo_ap, ncp,
                         wslice, xslice):
                """Emit 3*ncp DoubleRow matmuls accumulating into ps_out."""
                n = 0
                nt = 3 * ncp
                for xt, wt in ((x_hi_ap, w_hi_ap), (x_lo_ap, w_hi_ap),
                               (x_hi_ap, w_lo_ap)):
                    for c in range(ncp):
                        nc.tensor.matmul(
                            ps_out, wslice(wt, c), xslice(xt, c),
                            start=(n == 0), stop=(n == nt - 1), perf_mode=DR,
                        )
                        n += 1

            # ---------------- K projection: kT[e,s] ----------------
            # stream wk per eg (2 e-tiles); psum per e-tile [128, S]
            wkh_s = pa.tile([128, 2, NCK, 2, 256], E4, tag="wsh", name="wkh_s")
            wkl_s = pa.tile([128, 2, NCK, 2, 256], E5, tag="wsl", name="wkl_s")
            wvh_s = pa.tile([128, 2, NCK, 2, 512], E4, tag="wsh", name="wvh_s")
            wvl_s = pa.tile([128, 2, NCK, 2, 512], E5, tag="wsl", name="wvl_s")
            for eg in range(8):
                db = eg % 2
                nc.sync.dma_start(out=wkh_s[:, db], in_=wk_hi.ap()[:, eg])
                if eg == 0:
                    # first matmuls need only wk0-hi + the first xk chunks
                    nc.sync.dma_start(out=xkh_sb[:, 0:2],
                                      in_=xk_hi.ap()[:, 0:2])
                nc.sync.dma_start(out=wkl_s[:, db], in_=wk_lo.ap()[:, eg])
                # interleave the big x transfers in small chunks so they
                # don't wedge ahead of the streamed weight slices
                if eg == 0:
                    nc.sync.dma_start(out=xkh_sb[:, 2:4],
                                      in_=xk_hi.ap()[:, 2:4])
                    nc.sync.dma_start(out=xkl_sb[:, 0:2],
                                      in_=xk_lo.ap()[:, 0:2])
                    nc.sync.dma_start(out=xkl_sb[:, 2:4],
                                      in_=xk_lo.ap()[:, 2:4])
                    nc.sync.dma_start(out=bqk_sb, in_=bqk.ap())
                elif eg <= 2:
                    a = 2 * (eg - 1)
                    nc.sync.dma_start(out=xvh_sb[:, a:a + 2],
                                      in_=xv_hi.ap()[:, a:a + 2])
                    nc.sync.dma_start(out=xvl_sb[:, a:a + 2],
                                      in_=xv_lo.ap()[:, a:a + 2])
                elif eg <= 6:
                    src = (xq_hi, xq_lo)[(eg - 3) % 2]
                    dst = (xqh_sb, xql_sb)[(eg - 3) % 2]
                    a = 4 * ((eg - 3) // 2)
                    nc.sync.dma_start(out=dst[:, a:a + 4],
                                      in_=src.ap()[:, a:a + 4])
                elif eg == 7:
                    # prefetch the first V weight slice (slot bytes disjoint
                    # from the wk slots, so this does not wait on K)
                    nc.sync.dma_start(out=wvh_s[:, 1], in_=wv_hi.ap()[:, 0])
                    nc.sync.dma_start(out=wvl_s[:, 1], in_=wv_lo.ap()[:, 0])
                for t in range(2):
                    et = eg * 2 + t
                    ps = psA.tile([128, S], F32, tag="big", name=f"psk{et}")
                    for sh in range(2):
                        dr_terms(
                            ps[:, sh * 512:(sh + 1) * 512],
                            wkh_s, wkl_s, xkh_sb, xkl_sb, NCK,
                            lambda w, c, db=db, t=t: w[:, db, c, :,
                                                      t * 128:(t + 1) * 128],
                            lambda x, c, sh=sh: x[:, c, :,
                                                  sh * 512:(sh + 1) * 512],
                        )
                    nc.vector.tensor_scalar(
                        out=ktao_sb[:, :, et, :], in0=ps, scalar1=INV,
                        scalar2=bqk_sb[:, 1, et:et + 1], op0=MULT, op1=ADD,
                    )

            # ---------------- V projection: v[s, e] (no bias) ----------------
            # first Q weight slice staged in the (still idle) outst/aot slots
            wq0h = pa.tile([128, NCQ, 2, 256], E4, tag="outst", name="wq0h")
            wq0l = pa.tile([128, NCQ, 2, 256], E5, tag="aot", name="wq0l")
            for eq in range(4):
                db = (eq + 1) % 2
                if eq > 0:
                    nc.sync.dma_start(out=wvh_s[:, db], in_=wv_hi.ap()[:, eq])
                    nc.sync.dma_start(out=wvl_s[:, db], in_=wv_lo.ap()[:, eq])
                if eq == 1:
                    nc.sync.dma_start(out=wq0h, in_=wq_hi.ap()[:, 0])
                    nc.sync.dma_start(out=wq0l, in_=wq_lo.ap()[:, 0])
                for st in range(8):
                    # psB so V's pipeline does not wait on K's kT drains
                    ps = psB.tile([128, 512], F32, tag="sc", name=f"psv{eq}_{st}")
                    # roles swapped vs K/Q: stationary = x chunk, moving = w
                    dr_terms(
                        ps, xvh_sb, xvl_sb, wvh_s, wvl_s, NCK,
                        lambda x, c, st=st: x[:, c, :,
                                             st * 128:(st + 1) * 128],
                        lambda w, c, db=db: w[:, db, c],
                    )
                    nc.vector.tensor_scalar_mul(
                        v4[:, st, 4 * eq:4 * eq + 4, 0:HD], ps, INV)

            # ---------------- merged Q projection + attention ----------------
            aoThi = [pa.tile([128, 4, H, 128], E4, tag=t, name=f"aoThi{j}")
                     for j, t in ((0, "xkh"), (1, "xkl"))]
            aoTlo = [pa.tile([128, 4, H, 128], E5, tag=t, name=f"aoTlo{j}")
                     for j, t in ((0, "xvh"), (1, "xvl"))]

            def proj_q(h):
                # slice s>=1 lives in slot (s+1)%2; prefetch the next slice a
                # full head-pair (~24us) before its first use.  Slice 0 was
                # staged in wq0h/wq0l during the V projection.
                s = h // 2
                if h % 2 == 0 and s + 1 < 8:
                    nc.sync.dma_start(out=wsh_sb[:, s % 2],
                                      in_=wq_hi.ap()[:, s + 1])
                    nc.sync.dma_start(out=wsl_sb[:, s % 2],
                                      in_=wq_lo.ap()[:, s + 1])
                t = h % 2
                if s == 0:
                    whi, wlo = wq0h, wq0l
                    wsl_fn = (lambda w, c, t=t:
                              w[:, c, :, t * 128:(t + 1) * 128])
                else:
                    whi, wlo = wsh_sb, wsl_sb
                    wsl_fn = (lambda w, c, db=(s + 1) % 2, t=t:
                              w[:, db, c, :, t * 128:(t + 1) * 128])
                ps = psA.tile([128, S], F32, tag="big", name=f"psq{h}")
                for sh in range(2):
                    dr_terms(
                        ps[:, sh * 512:(sh + 1) * 512],
                        whi, wlo, xqh_sb, xql_sb, NCQ,
                        wsl_fn,
                        lambda x, c, sh=sh: x[:, c, :, sh * 512:(sh + 1) * 512],
                    )
                nc.vector.tensor_scalar(
                    out=qT(h), in0=ps, scalar1=INV,
                    scalar2=bqk_sb[:, 0, h:h + 1], op0=MULT, op1=ADD,
                )

            def transpose_split(sq, hf):
                db = (sq + hf) % 2
                at = aot_sb[:, db]
                nc.sync.dma_start_transpose(
                    out=at,
                    in_=ktao_sb[:, sq, hf * 8:(hf + 1) * 8, :])
                nc.vector.tensor_copy(
                    out=aoThi[sq // 4][:, sq % 4, hf * 8:(hf + 1) * 8, :],
                    in_=at)
                nc.vector.tensor_sub(
                    aoTlo[sq // 4][:, sq % 4, hf * 8:(hf + 1) * 8, :],
                    at,
                    aoThi[sq // 4][:, sq % 4, hf * 8:(hf + 1) * 8, :])

            # out-projection weight streams: woh reuses the qT0/pq slot (last
            # read: AV(13)); wol reuses the p_sb slot (last read: AV(14)) —
            # qT1 carries P(15) until the epilogue AV(15)
            woh_sb = pa.tile([128, 2, NCE, 2, 512], E4, tag="qT0", name="woh")
            wol_sb = pa.tile([128, 2, NCE, 2, 512], E5, tag="P", name="wol")

            def wo_fetch(e2c):
                db = e2c % 2
                nc.sync.dma_start(out=woh_sb[:, db], in_=wo_hi.ap()[:, e2c])
                nc.sync.dma_start(out=wol_sb[:, db], in_=wo_lo.ap()[:, e2c])

            # P double-buffering through dead slots: odd heads <8 use the xv
            # slots (dead after V, handed to aoTlo only from iteration 8);
            # odd heads 9-13 use the qT0 slot (heads 0-7 scores done by
            # then); everything else uses the static p_sb.  Adjacent heads
            # thus always use different buffers (except 14/15), so exp(h)
            # never waits for AV(h-1)'s reads.
            pxv = [pa.tile([128, 4, S], F16, tag=t, name=f"pxv{j}")
                   for j, t in ((0, "xvh"), (1, "xvl"))]
            pq_sb = pa.tile([128, 8, S], F16, tag="qT0", name="pq_sb")
            # head 15's P reuses the wq stream slots (dead after proj_q(15)
            # in iteration 14) — p_sb still carries P(14) for the lagged
            # AV(14) at that point.
            pq2 = [pa.tile([128, 4, S], F16, tag=t, name=f"pq2_{j}")
                   for j, t in ((0, "wsh"), (1, "wsl"))]

            def P_ap(h, sk):
                if h % 2 == 1 and h < 8:
                    return pxv[sk // 4][:, sk % 4, :]
                if h % 2 == 1 and h <= 13:
                    return pq_sb[:, sk, :]
                if h == 15:
                    return pq2[sk // 4][:, sk % 4, :]
                return p_sb[:, sk, :]

            def av_block(g):
                # attention-weighted V (+ rowsum/16 via the 1/16 column)
                for sq in range(8):
                    op = psA.tile([128, 512], F32, tag="big",
                                  name=f"op{g}_{sq}")
                    for sk in range(8):
                        nc.tensor.matmul(
                            op[:, 0:VROW],
                            P_ap(g, sk)[:, sq * 128:(sq + 1) * 128],
                            v_sb[:, sk, g * VROW:(g + 1) * VROW],
                            start=(sk == 0), stop=(sk == 7),
                        )
                    recip = small.tile([128, 1], F32, tag="recip")
                    nc.vector.reciprocal(out=recip, in_=op[:, HD:VROW])
                    nc.vector.tensor_scalar_mul(
                        ao(g, sq), op[:, 0:HD], recip)
                    if g == 15:
                        transpose_split(sq, 1)
                # heads 0-7 half: one transpose+split per iteration, after
                # the P reads of this block are done (the split's aoTlo
                # write reuses the pxv slot bytes)
                if 7 <= g <= 14:
                    transpose_split(g - 7, 0)

            proj_q(0)
            for h in range(H):
                # scoresT[sk, sq] then P = exp on ACT
                for sk in range(8):
                    ssp = psB.tile([128, S], F32, tag="sc", name=f"ss{h}_{sk}")
                    for sh in range(2):
                        nc.tensor.matmul(
                            ssp[:, sh * 512:(sh + 1) * 512],
                            kT(h, sk),
                            qT(h)[:, sh * 512:(sh + 1) * 512],
                            start=True, stop=True,
                        )
                    nc.scalar.activation(
                        out=P_ap(h, sk), in_=ssp, func=ACT_EXP,
                        bias=0.0, scale=1.0,
                    )
                # next head's Q projection overlaps the exp
                if h + 1 < H:
                    proj_q(h + 1)
                # AV lags one head so exp(h) has a full iteration to finish
                if h >= 1:
                    av_block(h - 1)
                if h == 15:
                    # after av_block(14): wol overwrites p_sb (= P(14))
                    wo_fetch(0)
            av_block(15)

            # ---------------- out projection ----------------
            for e2c in range(4):
                db = e2c % 2
                if e2c + 1 < 4:
                    wo_fetch(e2c + 1)
                for sq in range(8):
                    ps = psA.tile([128, 512], F32, tag="big",
                                  name=f"pso{e2c}_{sq}")
                    n = 0
                    for at, wt in ((aoThi, woh_sb), (aoTlo, woh_sb),
                                   (aoThi, wol_sb)):
                        for c in range(NCE):
                            nc.tensor.matmul(
                                ps,
                                at[sq // 4][:, sq % 4, 2 * c:2 * c + 2, :],
                                wt[:, db, c],
                                start=(n == 0), stop=(n == 23),
                                perf_mode=DR,
                            )
                            n += 1
                    ob = outst_sb[:, sq % 2]
                    nc.scalar.activation(
                        out=ob, in_=ps, func=ACT_IDENT, bias=0.0,
                        scale=INV,
                    )
                    nc.scalar.dma_start(
                        out=out.ap()[sq * 128:(sq + 1) * 128,
                                     e2c * 512:(e2c + 1) * 512],
                        in_=ob,
                    )

    nc.compile()
    return nc


def _get_nc():
    if "nc" not in _CACHED:
        _CACHED["nc"] = _build()
    return _CACHED["nc"]


def _split8(a, scale):
    """a (f32) -> (hi e4m3, lo e5m2) of a*scale."""
    s = (a * np.float32(scale)).astype(np.float32)
    hi = s.astype(ML_E4)
    lo = (s - hi.astype(np.float32)).astype(ML_E5)
    return hi, lo


def _wlayout(wT, npair, nslice, width):
    """[K, E] -> [128, nslice, npair, 2, width] stream layout."""
    K, Eo = wT.shape
    w = wT.reshape(npair, 2, 128, nslice, width)
    return np.ascontiguousarray(w.transpose(2, 3, 0, 1, 4))


def _xlayout(xT, npair):
    """[K, S] -> [128, npair, 2, S]."""
    K, Sx = xT.shape
    x = xT.reshape(npair, 2, 128, Sx)
    return np.ascontiguousarray(x.transpose(2, 0, 1, 3))


def _prepare_in_maps(inputs):
    query = np.asarray(inputs["query"], dtype=np.float32)
    key = np.asarray(inputs["key"], dtype=np.float32)
    value = np.asarray(inputs["value"], dtype=np.float32)
    Wq = np.asarray(inputs["Wq"], dtype=np.float32)
    bq = np.asarray(inputs["bq"], dtype=np.float32)
    Wk = np.asarray(inputs["Wk"], dtype=np.float32)
    bk = np.asarray(inputs["bk"], dtype=np.float32)
    Wv = np.asarray(inputs["Wv"], dtype=np.float32)
    Wo = np.asarray(inputs["Wo"], dtype=np.float32)

    scale_hd = np.float32(HD ** -0.5)
    wq_hi, wq_lo = _split8(Wq.T * scale_hd, WS)
    wk_hi, wk_lo = _split8(Wk.T, WS)
    wv_hi, wv_lo = _split8(Wv.T, WS)
    wo_hi, wo_lo = _split8(Wo.T, WS)
    wmaps = {
        "wq_hi": _wlayout(wq_hi, NCQ, 8, 256),
        "wq_lo": _wlayout(wq_lo, NCQ, 8, 256),
        "wk_hi": _wlayout(wk_hi, NCK, 8, 256),
        "wk_lo": _wlayout(wk_lo, NCK, 8, 256),
        "wv_hi": _wlayout(wv_hi, NCK, 4, 512),
        "wv_lo": _wlayout(wv_lo, NCK, 4, 512),
        "wo_hi": _wlayout(wo_hi, NCE, 4, 512),
        "wo_lo": _wlayout(wo_lo, NCE, 4, 512),
    }
    bqk = np.ascontiguousarray(
        np.stack([(bq * scale_hd), bk]).reshape(2, H, 128).transpose(2, 0, 1)
    ).astype(np.float32)

    in_maps = []
    for b in range(B):
        xqh, xql = _split8(query[b].T, XS)
        xkh, xkl = _split8(key[b].T, XS)
        xvh, xvl = _split8(value[b].T, XS)
        m = {
            "xq_hi": _xlayout(xqh, NCQ), "xq_lo": _xlayout(xql, NCQ),
            "xk_hi": _xlayout(xkh, NCK), "xk_lo": _xlayout(xkl, NCK),
            "xv_hi": _xlayout(xvh, NCK), "xv_lo": _xlayout(xvl, NCK),
            "bqk": bqk,
        }
        m.update(wmaps)
        in_maps.append(m)
    return in_maps


def run_on_device(inputs, **spmd_kwargs):
    """Run the bass kernel; returns (out [B,S,E] f32, BassKernelResults)."""
    in_maps = _prepare_in_maps(inputs)
    bo = np.asarray(inputs["bo"], dtype=np.float32)
    bv = np.asarray(inputs["bv"], dtype=np.float32)
    Wo = np.asarray(inputs["Wo"], dtype=np.float32)
    # softmax weights sum to 1, so the v bias shifts ao directly:
    # out = (ao + bv) @ Wo.T + bo
    bo_eff = bo + bv @ Wo.T
    res = run_bass_kernel_spmd(_get_nc(), in_maps,
                               core_ids=list(range(B)), **spmd_kwargs)
    out = np.stack([res.results[b]["out"] for b in range(B)], axis=0)
    return (out + bo_eff).astype(np.float32), res


def _numpy_reference(query, key, value, attention_mask,
                     Wq, bq, Wk, bk, Wv, bv, Wo, bo):
    # general fallback (only used when attention_mask isn't all ones)
    Bb, SQ, _ = query.shape
    SK = key.shape[1]
    q = query @ Wq.T + bq
    k = key @ Wk.T + bk
    v = value @ Wv.T + bv
    q = q.reshape(Bb, SQ, H, HD).transpose(0, 2, 1, 3)
    k = k.reshape(Bb, SK, H, HD).transpose(0, 2, 1, 3)
    v = v.reshape(Bb, SK, H, HD).transpose(0, 2, 1, 3)
    scores = np.einsum("bhqd,bhkd->bhqk", q, k) * (HD ** -0.5)
    scores = np.where(attention_mask[:, None, :, :] == 0,
                      np.float32(-1e10), scores)
    scores -= scores.max(-1, keepdims=True)
    p = np.exp(scores)
    p /= p.sum(-1, keepdims=True)
    o = np.einsum("bhqk,bhkd->bhqd", p, v)
    o = o.transpose(0, 2, 1, 3).reshape(Bb, SQ, E)
    return (o @ Wo.T + bo).astype(np.float32)


def kernel(**inputs):
    mask = np.asarray(inputs["attention_mask"])
    if not mask.all():
        return _numpy_reference(
            np.asarray(inputs["query"], dtype=np.float32),
            np.asarray(inputs["key"], dtype=np.float32),
            np.asarray(inputs["value"], dtype=np.float32), mask,
            np.asarray(inputs["Wq"], dtype=np.float32),
            np.asarray(inputs["bq"], dtype=np.float32),
            np.asarray(inputs["Wk"], dtype=np.float32),
            np.asarray(inputs["bk"], dtype=np.float32),
            np.asarray(inputs["Wv"], dtype=np.float32),
            np.asarray(inputs["bv"], dtype=np.float32),
            np.asarray(inputs["Wo"], dtype=np.float32),
            np.asarray(inputs["bo"], dtype=np.float32))
    out, _ = run_on_device(inputs)
    return out
